# revision 1
# baseline (speedup 1.0000x reference)
"""Differential attention (Marlin) TRN2 Bass kernel, sharded over heads on 8 cores.

Problem shapes (hardcoded): q1/q2 [1,16,2048,128] f32, k1/k2/v [1,4,2048,128] f32,
lambda_log [1] f32.  out = softmax(q1 k1^T/sqrt(D)) v - exp(lambda_log) * softmax(q2 k2^T/sqrt(D)) v.

Sharding: core c handles query heads {2c, 2c+1}, which share kv head c//2.
Each core gets q shards [2,S,D], k/v shards [S,D]; no cross-core communication.

Per-core algorithm (per head h, branch b):
  - Build Q^T, K^T ([D,S] layout) via PE transposes of [128,128] tiles
    (batched 4 per PSUM tile, single DVE copy out).
  - S^T[k,q] tiles = matmul(lhsT=K^T chunk, rhs=Q^T chunk) in float32r (full PE rate).
  - P^T = exp(S^T / sqrt(D)) on ScalarE, PSUM -> SBUF bf16.
  - O^T[d,q] += matmul(lhsT=V chunk (bf16), rhs=P^T chunk) accumulated in PSUM.
  - denominators: bf16 chunk-accumulation of P^T on DVE, then tiny N=1 matmuls
    (lhsT=acc slice, rhs=ones) give r in [q,1] layout directly.
  - PE-transpose O^T back to [q,d]; DVE normalizes and combines the two branches
    with per-partition scalars 1/r1 and -lam/r2; DMA out.

PSUM budget (8 banks): st 3x2 (MM1/exp pipeline; transpose staging and the
denominator tile borrow slots) + ot 2 (O^T accum, half-alternating) = 8.
"""

import math

import numpy as np

S = 2048
D = 128
NH = 2  # query heads per core
GK = 2  # k-chunks per exp group
SCALE = 1.0 / math.sqrt(D)

_CACHE = {}


def _build_nc(s=S, reps=1):
    import concourse.bass as bass  # noqa: F401
    import concourse.mybir as mybir
    from concourse import bacc
    from concourse.masks import make_identity
    from concourse.tile import TileContext

    f32 = mybir.dt.float32
    f32r = mybir.dt.float32r
    bf16 = mybir.dt.bfloat16
    Exp = mybir.ActivationFunctionType.Exp
    mult = mybir.AluOpType.mult
    add = mybir.AluOpType.add

    kc = s // 128
    nqc = max(1, s // 512)
    qcw = min(512, s)  # q-chunk width
    ntiles = s // 128

    nc = bacc.Bacc()
    q1 = nc.declare_dram_parameter("q1", [NH, s, D], f32, isOutput=False)
    q2 = nc.declare_dram_parameter("q2", [NH, s, D], f32, isOutput=False)
    k1 = nc.declare_dram_parameter("k1", [s, D], f32, isOutput=False)
    k2 = nc.declare_dram_parameter("k2", [s, D], f32, isOutput=False)
    v = nc.declare_dram_parameter("v", [s, D], f32, isOutput=False)
    lam_in = nc.declare_dram_parameter("lambda_log", [1], f32, isOutput=False)
    out = nc.declare_dram_parameter("out", [NH, s, D], f32, isOutput=True)

    qs = [q1, q2]
    ks = [k1, k2]

    with TileContext(nc) as tc:
        with (
            tc.tile_pool(name="persist", bufs=1) as pp,
            tc.tile_pool(name="stage", bufs=4) as stp,
            tc.tile_pool(name="qt", bufs=2) as qtp,
            tc.tile_pool(name="pt", bufs=8) as ptp,
            tc.tile_pool(name="accp", bufs=3) as accp,
            tc.tile_pool(name="otsb", bufs=3) as otsbp,
            tc.tile_pool(name="sden", bufs=3) as sdp,
            tc.tile_pool(name="ep", bufs=4) as epp,
            tc.tile_pool(name="ps_st", bufs=3, space="PSUM") as pst,
            tc.tile_pool(name="ps_ot", bufs=1, space="PSUM") as pot,
        ):
            # ---- constants / lambda ----
            ident = pp.tile([128, 128], f32, tag="ident")
            make_identity(nc, ident[:])
            ones_bf = pp.tile([128, 1], bf16, tag="ones_bf")
            nc.vector.memset(ones_bf[:], 1.0)
            ones_row = pp.tile([1, 128], f32, tag="ones_row")
            nc.vector.memset(ones_row[:], 1.0)

            lam_sb = pp.tile([1, 1], f32, tag="lam_sb")
            nc.sync.dma_start(lam_sb[0:1, :], lam_in[:].rearrange("(o a) -> o a", o=1))
            lam_e = pp.tile([1, 1], f32, tag="lam_e")
            nc.scalar.activation(lam_e[0:1, :], lam_sb[0:1, :], Exp)
            lam_ps = pst.tile([128, GK * qcw], f32, tag="st")
            nc.tensor.matmul(
                lam_ps[:, 0:1], ones_row[0:1, :], lam_e[0:1, :], start=True, stop=True
            )
            lam_bc = pp.tile([128, 1], f32, tag="lam_bc")
            nc.vector.tensor_copy(lam_bc[:], lam_ps[:, 0:1])

            # ---- staged input DMA (split into 4-tile blocks so the first
            # transpose batch can start after ~1/4 of the data lands) ----
            def dma_stage(src):
                st_tile = stp.tile([128, s], f32, tag="stage")
                src3 = src.rearrange("(t p) d -> p t d", p=128)
                dst3 = st_tile[:].rearrange("p (t d) -> p t d", d=D)
                for t0 in range(0, ntiles, 4):
                    nb = min(4, ntiles - t0)
                    nc.sync.dma_start(
                        dst3[:, t0 : t0 + nb, :], src3[:, t0 : t0 + nb, :]
                    )
                return st_tile

            # transpose staged [s-rows, d] -> dst [d, s] via batched PE transposes
            def build_T_fillers(dst, st_tile):
                def mk(t0):
                    def f():
                        tr = pst.tile([128, GK * qcw], f32, tag="st")
                        nb = min(4, ntiles - t0)
                        for j in range(nb):
                            t = t0 + j
                            nc.tensor.transpose(
                                tr[:, j * 128 : (j + 1) * 128],
                                st_tile[:, t * 128 : (t + 1) * 128],
                                ident[:],
                            )
                        nc.vector.tensor_copy(
                            dst[:, t0 * 128 : (t0 + nb) * 128], tr[:, 0 : nb * 128]
                        )

                    return f

                return [mk(t0) for t0 in range(0, ntiles, 4)]

            def build_T(dst, st_tile):
                for f in build_T_fillers(dst, st_tile):
                    f()

            # ---- V: load, convert to bf16 ----
            vst = dma_stage(v[:])
            v_bf = pp.tile([128, s], bf16, tag="v_bf")
            nc.vector.tensor_copy(v_bf[:], vst[:])

            # ---- K^T ----
            kts = []
            for b in range(2):
                kt = pp.tile([128, s], f32r, tag=f"kt{b}")
                build_T(kt, dma_stage(ks[b][:]))
                kts.append(kt)

            def build_qt(staged):
                qt = qtp.tile([128, s], f32r, tag="qt")
                build_T(qt, staged)
                return qt

            def attn_branch(qt, ktv, s_den, fillers=None):
                """One (head, branch): returns OT sbuf tile [128(d), s] fp32."""
                fillers = list(fillers or [])
                ot_sb = otsbp.tile([128, s], f32, tag="ot_sb")
                ot_dbuf = pot.tile([128, 2 * qcw], f32, tag="ot")
                for qc in range(nqc):
                    qsl = slice(qc * qcw, (qc + 1) * qcw)
                    half = qc % 2
                    ot_ps = ot_dbuf[:, half * qcw : (half + 1) * qcw]
                    acc = accp.tile([128, GK * qcw], bf16, tag="acc")
                    ngroups = kc // GK
                    for g in range(ngroups):
                        st = pst.tile([128, GK * qcw], f32, tag="st")
                        for i in range(GK):
                            ck = g * GK + i
                            nc.tensor.matmul(
                                st[:, i * qcw : (i + 1) * qcw],
                                ktv[:, ck * 128 : (ck + 1) * 128],
                                qt[:, qsl],
                                start=True,
                                stop=True,
                            )
                        pt = ptp.tile([128, GK * qcw], bf16, tag="pt")
                        nc.scalar.activation(pt[:], st[:], Exp, scale=SCALE)
                        for i in range(GK):
                            ck = g * GK + i
                            nc.tensor.matmul(
                                ot_ps,
                                v_bf[:, ck * 128 : (ck + 1) * 128],
                                pt[:, i * qcw : (i + 1) * qcw],
                                start=(ck == 0),
                                stop=(ck == kc - 1),
                            )
                        if g == 0:
                            nc.vector.tensor_copy(acc[:], pt[:])
                        else:
                            nc.vector.tensor_tensor(acc[:], acc[:], pt[:], add)
                        if fillers and (qc * ngroups + g) % 2 == 1:
                            fillers.pop(0)()
                    # fold GK sub-chunks to one [128, qcw]
                    racc = accp.tile([128, qcw], bf16, tag="racc")
                    nc.vector.tensor_tensor(
                        racc[:], acc[:, 0:qcw], acc[:, qcw : 2 * qcw], add
                    )
                    # denominators for this q-chunk: r[q] via N=1 matmuls
                    rq = pst.tile([128, GK * qcw], f32, tag="st")
                    nqt = qcw // 128
                    for t in range(nqt):
                        nc.tensor.matmul(
                            rq[:, t : t + 1],
                            racc[:, t * 128 : (t + 1) * 128],
                            ones_bf[:],
                            start=True,
                            stop=True,
                        )
                    nc.vector.reciprocal(
                        s_den[:, qc * nqt : qc * nqt + nqt], rq[:, 0:nqt]
                    )
                    # copy accumulated O^T chunk to SBUF
                    nc.vector.tensor_copy(ot_sb[:, qsl], ot_ps)
                for f in fillers:
                    f()
                return ot_sb

            def epilogue_fillers(h, ot1, ot2, s1, s2):
                s2l = sdp.tile([128, ntiles], f32, tag="s2l")

                def head():
                    nc.vector.tensor_scalar(
                        s2l[:], s2[:], lam_bc[:, 0:1], -1.0, mult, mult
                    )

                def mk(t0):
                    # 2 output q-tiles per tr slot: [o1(t), o1(t+1), o2(t), o2(t+1)]
                    def f():
                        tr = pst.tile([128, GK * qcw], f32, tag="st")
                        for j in range(2):
                            tsl = slice((t0 + j) * 128, (t0 + j + 1) * 128)
                            nc.tensor.transpose(
                                tr[:, j * 128 : (j + 1) * 128], ot1[:, tsl], ident[:]
                            )
                            nc.tensor.transpose(
                                tr[:, (2 + j) * 128 : (3 + j) * 128],
                                ot2[:, tsl],
                                ident[:],
                            )
                        for j in range(2):
                            t = t0 + j
                            tsl = slice(t * 128, (t + 1) * 128)
                            t1 = epp.tile([128, 128], f32, tag="t1")
                            nc.vector.tensor_scalar_mul(
                                t1[:], tr[:, j * 128 : (j + 1) * 128], s1[:, t : t + 1]
                            )
                            o = epp.tile([128, 128], f32, tag="o")
                            nc.vector.scalar_tensor_tensor(
                                o[:],
                                tr[:, (2 + j) * 128 : (3 + j) * 128],
                                s2l[:, t : t + 1],
                                t1[:],
                                mult,
                                add,
                            )
                            nc.sync.dma_start(out[h, tsl, :], o[:])

                    return f

                return [head] + [mk(t0) for t0 in range(0, ntiles, 2)]

            def epilogue(h, ot1, ot2, s1, s2):
                for f in epilogue_fillers(h, ot1, ot2, s1, s2):
                    f()

            # ---- main schedule ----
            def schedule():
                q_staged = dma_stage(qs[0][0])
                qt_cur = build_qt(q_staged)
                nxt_staged = dma_stage(qs[1][0])
                prev = None  # (h, ot1, ot2, s1, s2) pending epilogue
                for h in range(NH):
                    ots = []
                    sds = []
                    for b in range(2):
                        fillers = []
                        # next QT build interleaves with this branch's groups
                        nxt = (h, b + 1) if b == 0 else (h + 1, 0)
                        qt_nxt = None
                        if nxt[0] < NH:
                            qt_nxt = qtp.tile([128, s], f32r, tag="qt")
                            fillers += build_T_fillers(qt_nxt, nxt_staged)
                        if prev is not None:
                            fillers += epilogue_fillers(*prev)
                            prev = None
                        s_den = sdp.tile([128, ntiles], f32, tag=f"sden{b}")
                        ot = attn_branch(qt_cur, kts[b], s_den, fillers)
                        ots.append(ot)
                        sds.append(s_den)
                        if nxt[0] < NH:
                            qt_cur = qt_nxt
                            nxt2 = (
                                (nxt[0], nxt[1] + 1)
                                if nxt[1] == 0
                                else (nxt[0] + 1, 0)
                            )
                            if nxt2[0] < NH:
                                nxt_staged = dma_stage(qs[nxt2[1]][nxt2[0]])
                    prev = (h, ots[0], ots[1], sds[0], sds[1])
                epilogue(*prev)

            if reps == 1:
                schedule()
            else:
                with tc.For_i(0, reps, 1):
                    schedule()

    nc.compile()
    return nc


def _shard_inputs(inputs):
    q1 = np.asarray(inputs["q1"], dtype=np.float32)
    q2 = np.asarray(inputs["q2"], dtype=np.float32)
    k1 = np.asarray(inputs["k1"], dtype=np.float32)
    k2 = np.asarray(inputs["k2"], dtype=np.float32)
    v = np.asarray(inputs["v"], dtype=np.float32)
    lam = np.asarray(inputs["lambda_log"], dtype=np.float32).reshape(1)
    in_maps = []
    for c in range(8):
        kv = c // 2
        in_maps.append(
            {
                "q1": np.ascontiguousarray(q1[0, 2 * c : 2 * c + 2]),
                "q2": np.ascontiguousarray(q2[0, 2 * c : 2 * c + 2]),
                "k1": np.ascontiguousarray(k1[0, kv]),
                "k2": np.ascontiguousarray(k2[0, kv]),
                "v": np.ascontiguousarray(v[0, kv]),
                "lambda_log": lam,
            }
        )
    return in_maps


def kernel(q1, k1, v, q2, k2, lambda_log):
    from concourse.bass_utils import run_bass_kernel_spmd

    inputs = {
        "q1": q1,
        "k1": k1,
        "v": v,
        "q2": q2,
        "k2": k2,
        "lambda_log": lambda_log,
    }
    in_maps = _shard_inputs(inputs)
    if "nc" not in _CACHE:
        _CACHE["nc"] = _build_nc()
    nc = _CACHE["nc"]
    res = run_bass_kernel_spmd(nc, in_maps, core_ids=list(range(8)))
    outs = np.stack([res.results[c]["out"] for c in range(8)])  # [8, 2, S, D]
    return outs.reshape(1, 16, S, D).astype(np.float32)


# ---------------------------------------------------------------------------
# Timing helpers (used by test.py; not needed for grading correctness)
# ---------------------------------------------------------------------------
def _make_runner(nc, n_cores=8):
    """Persistent jitted SPMD runner with device-resident inputs."""
    import jax
    import jax.numpy as jnp
    import concourse.mybir as mybir
    from concourse.bass2jax import (
        _bass_exec_p,
        install_neuronx_cc_hook,
        partition_id_tensor,
    )
    from jax.sharding import Mesh, NamedSharding, PartitionSpec
    from jax.experimental.shard_map import shard_map

    install_neuronx_cc_hook()
    partition_name = nc.partition_id_tensor.name if nc.partition_id_tensor else None
    in_names, out_names, out_avals, zero_outs = [], [], [], []
    for alloc in nc.m.functions[0].allocations:
        if not isinstance(alloc, mybir.MemoryLocationSet):
            continue
        name = alloc.memorylocations[0].name
        if alloc.kind == "ExternalInput":
            if name != partition_name:
                in_names.append(name)
        elif alloc.kind == "ExternalOutput":
            out_names.append(name)
            out_avals.append(
                jax.core.ShapedArray(
                    tuple(alloc.tensor_shape), mybir.dt.np(alloc.dtype)
                )
            )
            zero_outs.append(
                np.zeros(tuple(alloc.tensor_shape), mybir.dt.np(alloc.dtype))
            )
    n_params, n_outs = len(in_names), len(out_avals)
    all_in_names = (
        list(in_names) + list(out_names) + ([partition_name] if partition_name else [])
    )

    def _body(*args):
        ins = list(args[:n_params])
        outs = list(args[n_params:])
        operands = ins + outs + ([partition_id_tensor()] if partition_name else [])
        return tuple(
            _bass_exec_p.bind(
                *operands,
                out_avals=tuple(out_avals),
                in_names=tuple(all_in_names),
                out_names=tuple(out_names),
                lowering_input_output_aliases=(),
                sim_require_finite=True,
                sim_require_nnan=True,
                nc=nc,
            )
        )

    devices = jax.devices()[:n_cores]
    mesh = Mesh(np.asarray(devices), ("core",))
    sh = NamedSharding(mesh, PartitionSpec("core"))
    donate = tuple(range(n_params, n_params + n_outs))
    sharded = jax.jit(
        shard_map(
            _body,
            mesh=mesh,
            in_specs=(PartitionSpec("core"),) * (n_params + n_outs),
            out_specs=(PartitionSpec("core"),) * n_outs,
            check_rep=False,
        ),
        donate_argnums=donate,
        keep_unused=True,
    )
    mkzeros = jax.jit(
        lambda: tuple(
            jnp.zeros((n_cores * z.shape[0], *z.shape[1:]), z.dtype)
            for z in zero_outs
        ),
        out_shardings=(sh,) * n_outs,
    )

    state = {}

    def run(in_maps):
        if "dev_in" not in state:
            concat_in = [
                np.concatenate(
                    [np.asarray(in_maps[c][n]) for c in range(n_cores)], axis=0
                )
                for n in in_names
            ]
            state["dev_in"] = [jax.device_put(a, sh) for a in concat_in]
        zs = mkzeros()
        out = sharded(*state["dev_in"], *zs)
        jax.block_until_ready(out)
        return [
            {
                n: np.asarray(out[i]).reshape(n_cores, *out_avals[i].shape)[c]
                for i, n in enumerate(out_names)
            }
            for c in range(n_cores)
        ]

    return run


def time_kernel(inputs, reps=(64, 256), calls=40, expected=None):
    """Estimated per-execution HW time in ns, via two on-device For_i loop
    lengths with alternating calls (cancels host/tunnel drift)."""
    import time as _time

    in_maps = _shard_inputs(inputs)
    rA, rB = reps
    ncA = _build_nc(reps=rA)
    ncB = _build_nc(reps=rB)
    runA = _make_runner(ncA)
    runB = _make_runner(ncB)
    resA = runA(in_maps)
    resB = runB(in_maps)
    if expected is not None:
        for nm, res in (("repsA", resA), ("repsB", resB)):
            outs = np.stack([res[c]["out"] for c in range(8)]).reshape(1, 16, S, D)
            rel = np.abs(outs - expected).max() / np.abs(expected).max()
            print(f"[time_kernel] {nm} loop-build rel err: {rel:.3g}")
    wA, wB = [], []
    for _ in range(calls):
        t0 = _time.perf_counter()
        runA(in_maps)
        t1 = _time.perf_counter()
        runB(in_maps)
        t2 = _time.perf_counter()
        wA.append(t1 - t0)
        wB.append(t2 - t1)
    per_iter = (min(wB) - min(wA)) / (rB - rA)
    print(
        f"[time_kernel] minA={min(wA)*1e3:.2f}ms minB={min(wB)*1e3:.2f}ms "
        f"({rA} vs {rB} iters) -> per-iter {per_iter*1e6:.1f}us"
    )
    return per_iter * 1e9

